# revision 12
# baseline (speedup 1.0000x reference)
"""LoRA linear kernel for Trainium2, 8-core SPMD.

Computes out = x @ W^T + bias + (alpha/r) * (x @ A^T) @ B^T for
x [4, 4096, 4096], W [4096, 4096], A [16, 4096], B [4096, 16].

Sharding: data-parallel over tokens — each of the 8 cores owns 2048 of the
16384 flattened (B*S) tokens and the full output width.

Per-core kernel:
  - x shard cached entirely in SBUF as bf16 [128(kin), 32(ko), 2048(t)]
    (contraction dim on partitions), loaded once.
  - W streamed as bf16 [128, 16, 512] half-k column tiles, double-buffered
    (two half-k tiles = one full-k 512-wide column block).
  - out tile [128 t, 512 o]: 32 accumulating N=512 matmuls over k plus one
    extra matmul adding the LoRA term (rank 16 zero-padded to 128), all in
    one fp32 PSUM accumulation group.
  - xa^T = A @ x_shard^T computed on-device first ([16, 2048] fp32 -> bf16).
  - bias fused into the PSUM->SBUF eviction on VectorE; fp32 output.
"""

import sys

for _p in ("/opt/trn_rl_repo", "/root/.axon_site/_ro/trn_rl_repo"):
    if _p not in sys.path:
        sys.path.append(_p)

import numpy as np
from ml_dtypes import bfloat16

import concourse.bass as bass
import concourse.mybir as mybir
import concourse.tile as tile
from concourse import bacc
from concourse.bass import ts
from concourse.bass_utils import run_bass_kernel_spmd

# Problem shapes (hardcoded per contract)
B, S, D_IN, D_OUT = 4, 4096, 4096, 4096
R = 16
SCALE = 16.0 / 16.0
T = B * S                 # 16384 tokens
NCORES = 8
TC = T // NCORES          # 2048 tokens per core
P = 128
KO = D_IN // P            # 32 k-subtiles
KQ = 8                    # k-subtiles per W quarter tile
NQ = KO // KQ             # 4 quarter tiles per column block
NT = 512                  # output-column tile width
NNT = D_OUT // NT         # 8 n tiles
MS = TC // P              # 16 m subtiles per core

_BF = mybir.dt.bfloat16
_F32 = mybir.dt.float32


def _build_nc(repeat=1):
    nc = bacc.Bacc("TRN2", target_bir_lowering=False, debug=False,
                   num_devices=NCORES)

    xc_d = nc.dram_tensor("xc", [P, KO, TC], _BF, kind="ExternalInput").ap()
    w_d = nc.dram_tensor("wt", [NNT, NQ, P, KQ, NT], _BF,
                         kind="ExternalInput").ap()
    bias_d = nc.dram_tensor("biasr", [P, D_OUT], _F32, kind="ExternalInput").ap()
    at_d = nc.dram_tensor("at", [P, KO, R], _BF, kind="ExternalInput").ap()
    btp_d = nc.dram_tensor("btp", [P, D_OUT], _BF, kind="ExternalInput").ap()
    out_d = nc.dram_tensor("out", [TC, D_OUT], _F32, kind="ExternalOutput").ap()

    with tile.TileContext(nc) as tc:
        with (
            tc.tile_pool(name="xpool", bufs=1) as xpool,
            tc.tile_pool(name="wpool", bufs=5) as wpool,
            tc.tile_pool(name="cpool", bufs=1) as cpool,
            tc.tile_pool(name="bpool", bufs=2) as bpool,
            tc.tile_pool(name="opool", bufs=2) as opool,
            tc.tile_pool(name="psum", bufs=6, space="PSUM") as pp,
            tc.tile_pool(name="psum_xa", bufs=2, space="PSUM") as pxa,
        ):
            for _ in range(repeat):
                # --- load x shard into SBUF as 16 independent chunk tiles
                # (2 k-subtiles each) so consumers only wait for the chunk
                # they read, not the whole 16.8 MB load.
                at_sb = cpool.tile([P, KO, R], _BF)
                nc.sync.dma_start(at_sb[:], at_d[:])
                KC = 2                    # k-subtiles per x chunk
                xcs = []
                for ck in range(KO // KC):
                    xt = xpool.tile([P, KC, TC], _BF, tag=f"x{ck}")
                    nc.sync.dma_start(xt[:], xc_d[:, ts(ck, KC)])
                    xcs.append(xt)

                def xck(ko):
                    return xcs[ko // KC][:, ko % KC]
                btp_sb = cpool.tile([P, D_OUT], _BF)
                nc.sync.dma_start(btp_sb[:], btp_d[:])

                # --- xa^T = A @ x_shard^T : [16, 2048], rank rows padded
                xaT_sb = cpool.tile([P, TC], _BF)
                nc.any.memzero(xaT_sb[:])
                for tt in range(TC // 512):
                    ps = pxa.tile([P, 512], _F32)
                    for ko in range(KO):
                        nc.tensor.matmul(
                            ps[:R], at_sb[:, ko], xck(ko)[:, ts(tt, 512)],
                            start=(ko == 0), stop=(ko == KO - 1),
                        )
                    nc.vector.tensor_copy(xaT_sb[:R, ts(tt, 512)], ps[:R])

                # --- main loop: out[t, o] tiles
                for nt in range(NNT):
                    wq = []
                    for q in range(NQ):
                        w_sb = wpool.tile([P, KQ, NT], _BF, tag="w")
                        nc.sync.dma_start(w_sb[:], w_d[nt, q])
                        wq.append(w_sb)
                    bias_sb = bpool.tile([P, NT], _F32)
                    nc.sync.dma_start(bias_sb[:], bias_d[:, ts(nt, NT)])
                    for ms in range(MS):
                        ps = pp.tile([P, NT], _F32)
                        for ko in range(KO):
                            nc.tensor.matmul(
                                ps[:],
                                xck(ko)[:, ts(ms, P)],
                                wq[ko // KQ][:, ko % KQ],
                                start=(ko == 0), stop=False,
                            )
                        nc.tensor.matmul(
                            ps[:], xaT_sb[:, ts(ms, P)], btp_sb[:, ts(nt, NT)],
                            start=False, stop=True,
                        )
                        out_sb = opool.tile([P, NT], _F32)
                        nc.vector.tensor_add(out_sb[:], ps[:], bias_sb[:])
                        nc.sync.dma_start(out_d[ts(ms, P), ts(nt, NT)],
                                          out_sb[:])

    nc.compile()
    return nc


_NC_CACHE = None


def _get_nc():
    global _NC_CACHE
    if _NC_CACHE is None:
        _NC_CACHE = _build_nc()
    return _NC_CACHE


def _prep_inputs(x, weight, bias, lora_A, lora_B):
    xr = np.ascontiguousarray(x.reshape(T, D_IN))
    wt = np.ascontiguousarray(
        weight.reshape(NNT, NT, NQ, KQ, P).transpose(0, 2, 4, 3, 1)
    ).astype(bfloat16)
    biasr = np.ascontiguousarray(
        np.broadcast_to(bias.astype(np.float32), (P, D_OUT))
    )
    at = np.ascontiguousarray(
        lora_A.reshape(R, KO, P).transpose(2, 1, 0)
    ).astype(bfloat16)
    btp = np.zeros((P, D_OUT), dtype=bfloat16)
    btp[:R] = (SCALE * lora_B.T.astype(np.float32)).astype(bfloat16)

    in_maps = []
    for c in range(NCORES):
        xc = np.ascontiguousarray(
            xr[c * TC:(c + 1) * TC].reshape(TC, KO, P).transpose(2, 1, 0)
        ).astype(bfloat16)
        in_maps.append(
            {"xc": xc, "wt": wt, "biasr": biasr, "at": at, "btp": btp}
        )
    return in_maps


def run(inputs, trace=False):
    nc = _get_nc()
    in_maps = _prep_inputs(**inputs)
    res = run_bass_kernel_spmd(nc, in_maps, list(range(NCORES)), trace=trace)
    out = np.concatenate([r["out"] for r in res.results], axis=0)
    return out.reshape(B, S, D_OUT), res


def kernel(**inputs):
    out, _ = run(inputs, trace=False)
    return out
